# revision 6
# baseline (speedup 1.0000x reference)
"""KANLinear forward on 8 Trainium2 NeuronCores (data-parallel over tokens).

Math: out = silu(x) @ Wb.T + bb + ss * (einsum('oib,nib->no', Ws, basis(tanh x)) + sb)
The cubic B-spline basis over the uniform 12-knot grid is rewritten exactly as
truncated powers r_m = relu(tanh(x) - c_m)^3, c_m = -1 + m*(2/11), m = 0..10,
with the 5-tap conv [1,-4,6,-4,1]/(6 h^3) folded into the spline weights on host.
Device: tanh/silu/square on ACT, relu via tensor_scalar, cube-mul on DVE,
12 accumulating float32r matmuls (K=128 each) per 512-token chunk on PE.
float32r keeps ~fp32 matmul precision at 1 cycle/row (measured 1.7e-4 rel).
"""
import sys
if "/opt/trn_rl_repo" not in sys.path:
    sys.path.insert(0, "/opt/trn_rl_repo")
import numpy as np
from contextlib import ExitStack

import concourse.bass as bass
import concourse.tile as tile
import concourse.mybir as mybir
from concourse import bacc
from concourse.bass_utils import run_bass_kernel_spmd

F32, F32R = mybir.dt.float32, mybir.dt.float32r

N_CORES = 8
N_TOK = 16 * 4096            # 65536 total tokens
TOK_C = N_TOK // N_CORES     # 8192 per core
TILE = 2048                  # pointwise tile (tokens)
CHUNK = 512                  # matmul free-dim chunk (one PSUM bank)
M = 11
H = 2.0 / 11.0
C_SHIFTS = [-1.0 + H * m for m in range(M)]

_CACHE = {}
LAST_EXEC_NS = None
LAST_PROFILE = None


def _build():
    if "nc" in _CACHE:
        return _CACHE["nc"]
    nc = bacc.Bacc(None, target_bir_lowering=False, debug=False)
    x_d = nc.declare_dram_parameter("xT", [128, TOK_C], F32, isOutput=False)
    wb_d = nc.declare_dram_parameter("wb", [128, 128], F32, isOutput=False)      # [i, o]
    ws_d = nc.declare_dram_parameter("ws", [128, M, 128], F32, isOutput=False)   # [i, m, o]
    bias_d = nc.declare_dram_parameter("bias", [128, 1], F32, isOutput=False)    # [o, 1]
    y_d = nc.declare_dram_parameter("yT", [128, TOK_C], F32, isOutput=True)      # [o, tok]

    Act = mybir.ActivationFunctionType
    Alu = mybir.AluOpType

    with tile.TileContext(nc) as tc, ExitStack() as ctx:
        const = ctx.enter_context(tc.tile_pool(name="const", bufs=1))
        xpool = ctx.enter_context(tc.tile_pool(name="x", bufs=2))
        tpool = ctx.enter_context(tc.tile_pool(name="t", bufs=2))
        spool = ctx.enter_context(tc.tile_pool(name="s", bufs=2))
        vpool = ctx.enter_context(tc.tile_pool(name="v", bufs=2))
        v2pool = ctx.enter_context(tc.tile_pool(name="v2", bufs=2))
        rpool = ctx.enter_context(tc.tile_pool(name="r", bufs=3))
        opool = ctx.enter_context(tc.tile_pool(name="o", bufs=4))
        psum = ctx.enter_context(tc.tile_pool(name="ps", bufs=2, space="PSUM"))

        # weights -> SBUF, round to f32r via DVE copy (f32r matmul wants
        # producers that round)
        wb_raw = const.tile([128, 128], F32)
        nc.sync.dma_start(out=wb_raw[:], in_=wb_d[:])
        ws_raw = const.tile([128, M, 128], F32)
        nc.sync.dma_start(out=ws_raw[:], in_=ws_d[:])
        bias_sb = const.tile([128, 1], F32)
        nc.sync.dma_start(out=bias_sb[:], in_=bias_d[:])

        # base + high-m spline features have low cancellation-amplification:
        # f32r (1 cyc/row) is safe there; low-m features need full fp32 (4 cyc/row)
        wb_sb = const.tile([128, 128], F32R)
        nc.vector.tensor_copy(wb_sb[:], wb_raw[:])
        w_m = []
        for m in range(M):
            if m >= 8:
                wt = const.tile([128, 128], F32R, tag=f"wm{m}", name=f"wm{m}")
                nc.vector.tensor_copy(wt[:], ws_raw[:, m, :])
                w_m.append(wt)
            else:
                w_m.append(ws_raw[:, m, :])

        for it in range(TOK_C // TILE):
            j0 = it * TILE
            x_sb = xpool.tile([128, TILE], F32)
            nc.sync.dma_start(out=x_sb[:], in_=x_d[:, j0:j0 + TILE])

            t_sb = tpool.tile([128, TILE], F32)
            nc.scalar.activation(t_sb[:], x_sb[:], Act.Tanh)
            s_sb = spool.tile([128, TILE], F32R)
            nc.scalar.activation(s_sb[:], x_sb[:], Act.Silu)

            nchunk = TILE // CHUNK
            ps_t = [psum.tile([128, CHUNK], F32, tag=f"psc{k}", name=f"ps_{it}_{k}") for k in range(nchunk)]
            for k in range(nchunk):
                nc.tensor.matmul(ps_t[k][:], wb_sb[:],
                                 s_sb[:, k * CHUNK:(k + 1) * CHUNK],
                                 start=True, stop=False)

            for m in range(M):
                v = vpool.tile([128, TILE], F32, tag="v")
                nc.vector.tensor_scalar(v[:], t_sb[:], C_SHIFTS[m], 0.0,
                                        Alu.subtract, Alu.max)
                v2 = v2pool.tile([128, TILE], F32, tag="v2")
                nc.scalar.activation(v2[:], v[:], Act.Square)
                r = rpool.tile([128, TILE], F32R if m >= 8 else F32, tag="rr" if m >= 8 else "r")
                nc.vector.tensor_mul(r[:], v[:], v2[:])
                for k in range(nchunk):
                    nc.tensor.matmul(ps_t[k][:], w_m[m][:],
                                     r[:, k * CHUNK:(k + 1) * CHUNK],
                                     start=False, stop=(m == M - 1))

            for k in range(nchunk):
                o_sb = opool.tile([128, CHUNK], F32, tag="o")
                nc.vector.tensor_scalar(o_sb[:], ps_t[k][:], bias_sb[:, 0:1], None,
                                        Alu.add)
                nc.sync.dma_start(out=y_d[:, j0 + k * CHUNK:j0 + (k + 1) * CHUNK],
                                  in_=o_sb[:])
    nc.finalize()
    _CACHE["nc"] = nc
    return nc


def _prep_weights(base_weight, spline_weight, base_bias, spline_bias, spline_scale):
    ss = float(np.asarray(spline_scale).reshape(-1)[0])
    sw = np.asarray(spline_weight, dtype=np.float64)          # [o, i, 8]
    d = np.array([1.0, -4.0, 6.0, -4.0, 1.0])
    Wt = np.zeros((128, M, 128), dtype=np.float64)            # [i, m, o]
    for m in range(M):
        for j in range(max(0, m - 4), min(7, m) + 1):
            Wt[:, m, :] += sw[:, :, j].T * d[m - j]
    Wt *= ss / (6.0 * H ** 3)
    wb = np.asarray(base_weight, dtype=np.float32).T.copy()   # [i, o]
    bias = (np.asarray(base_bias, dtype=np.float64)
            + ss * np.asarray(spline_bias, dtype=np.float64))
    return wb.astype(np.float32), Wt.astype(np.float32), \
        bias.astype(np.float32).reshape(128, 1)


def kernel(x, grid, base_weight, base_bias, spline_weight, spline_bias,
           spline_scale, **_unused):
    nc = _build()
    wb, ws, bias = _prep_weights(base_weight, spline_weight, base_bias,
                                 spline_bias, spline_scale)
    xf = np.ascontiguousarray(np.asarray(x, dtype=np.float32).reshape(N_TOK, 128))
    in_maps = []
    for c in range(N_CORES):
        shard = np.ascontiguousarray(xf[c * TOK_C:(c + 1) * TOK_C].T)  # [128, TOK_C]
        in_maps.append({"xT": shard, "wb": wb, "ws": ws, "bias": bias})
    import os
    trace = bool(int(os.environ.get("KAN_PROFILE", "0")))
    res = run_bass_kernel_spmd(nc, in_maps, list(range(N_CORES)), trace=trace)
    global LAST_EXEC_NS, LAST_PROFILE
    LAST_EXEC_NS = res.exec_time_ns
    LAST_PROFILE = res.profile_json
    out = np.concatenate([res.results[c]["yT"].T for c in range(N_CORES)], axis=0)
    return out.reshape(np.asarray(x).shape[:-1] + (128,)).astype(np.float32)


if __name__ == "__main__":
    rng = np.random.default_rng(0)
    ins = {
        "x": rng.standard_normal((16, 4096, 128)).astype(np.float32),
        "grid": np.tile(np.linspace(-1, 1, 12, dtype=np.float32), (128, 1)),
        "base_weight": (rng.standard_normal((128, 128)) * 0.1).astype(np.float32),
        "base_bias": np.zeros(128, np.float32),
        "spline_weight": (rng.standard_normal((128, 128, 8)) * 0.1).astype(np.float32),
        "spline_bias": np.zeros(128, np.float32),
        "spline_scale": np.ones(1, np.float32),
    }
    print(kernel(**ins).shape)
